# revision 53
# baseline (speedup 1.0000x reference)
"""GQA attention (B=2,S=2048,DIM=2048,H=32,KVH=8,HD=64) + RoPE, causal.

Distributed over 8 TRN2 NeuronCores: core = 4*batch + head_group.
Each core computes attention for its 8 q-heads (2 kv-heads) of one batch.
Q^T / K^T are produced directly by the projection matmuls (weights
stationary, x^T moving) so no transpose of Q/K is ever needed; RoPE is
applied in the transposed [hd, seq] layout with replicated cos/sin rows.
The causal mask is fused into the score matmul as an accumulated
(identity x lower-triangular -240) product.  The output projection is
computed per chunk as partial products against the core's own 512 rows
of wo, then summed + distributed with a per-chunk ReduceScatter.
Host-side work is layout-only: weight column/row permutations, batch
split, cos/sin row replication, and concatenation of per-core outputs.
"""
import numpy as np

import concourse.bass as bass
import concourse.bacc as bacc
import concourse.tile as tile
from concourse.tile import add_dep_helper
import concourse.mybir as mybir
from concourse import bass_utils


def _ensure_axon_hooks_shim():
    """bass_utils imports antenv.axon_hooks when BASS_TRACE is set; the
    module is absent in some images. Provide a no-op shim so tracing env
    vars cannot crash the run."""
    import sys, types
    try:
        import antenv  # noqa
        if "antenv.axon_hooks" in sys.modules:
            return
        import importlib
        try:
            importlib.import_module("antenv.axon_hooks")
            return
        except ImportError:
            pass
        mod = types.ModuleType("antenv.axon_hooks")
        mod._hook = None
        mod.get_axon_ntff_profile_hook = lambda: mod._hook

        def set_axon_ntff_profile_hook(h):
            mod._hook = h
        mod.set_axon_ntff_profile_hook = set_axon_ntff_profile_hook
        sys.modules["antenv.axon_hooks"] = mod
        antenv.axon_hooks = mod
    except Exception:
        pass


_ensure_axon_hooks_shim()

F32 = mybir.dt.float32
BF16 = mybir.dt.bfloat16

B, S, DIM = 2, 2048, 2048
H, KVH, HD = 32, 8, 64
N_CORES = 8
GROUPS = [[0, 1, 2, 3], [4, 5, 6, 7]]
NCH = 4            # sequence chunks (queries) of 512
CHUNK = S // NCH   # 512
SEQT = S // 128    # 16 seq tiles
DT = DIM // 128    # 16 contraction tiles
# q-head slot order inside a core: slot s holds local q-head s//2 + 4*(s%2),
# so slot parity == local kv-head index (kv = local_head // 4).
SLOT_TO_LOCAL = [s // 2 + 4 * (s % 2) for s in range(8)]
# rope pair permutation within one head: 16-interleaved halves so the
# (a, b) cross-swap is a within-32-quadrant partition shuffle:
# [a0..a15, b0..b15, a16..a31, b16..b31] where a_i = dim 2i, b_i = dim 2i+1
HD_PERM = np.concatenate([np.arange(0, 32, 2), np.arange(1, 32, 2),
                          np.arange(32, 64, 2), np.arange(33, 64, 2)])
SWAP_MASK = list(range(16, 32)) + list(range(0, 16))
MASK_NEG = -240.0


def _build():
    nc = bacc.Bacc("TRN2", target_bir_lowering=False, debug=False,
                   num_devices=N_CORES)
    x_d = nc.dram_tensor("x", [S, DIM], BF16, kind="ExternalInput")
    wq_d = nc.dram_tensor("wq", [DIM, 512], BF16, kind="ExternalInput")
    wkv_d = nc.dram_tensor("wkv", [DIM, 256], BF16, kind="ExternalInput")
    wo_d = nc.dram_tensor("wo", [512, DIM], BF16, kind="ExternalInput")
    cosr_d = nc.dram_tensor("cosr", [128, S], BF16, kind="ExternalInput")
    sinr_d = nc.dram_tensor("sinr", [128, S], BF16, kind="ExternalInput")
    out_d = nc.dram_tensor("out", [CHUNK, DIM], BF16, kind="ExternalOutput")

    Exp = mybir.ActivationFunctionType.Exp
    Copy = mybir.ActivationFunctionType.Copy

    with tile.TileContext(nc) as tc:
        with tc.tile_pool(name="dram", bufs=1, space="DRAM") as dram, \
             tc.tile_pool(name="wpool", bufs=1) as wpool:
            # ---- DRAM scratch ----
            # one tensor per chunk (and per tail piece) so the tile dep
            # tracker never serializes chunk c+1's partial-writes behind
            # chunk c's ReduceScatter read of a shared tensor
            partials = [dram.tile([CHUNK, DIM], BF16, name=f"partial{i}",
                                  tag=f"partial{i}")
                        for i in range(NCH - 1)]
            rsouts = [dram.tile([128, DIM], BF16, name=f"rsout{i}",
                                tag=f"rsout{i}")
                      for i in range(NCH - 1)]
            piece_partials = [dram.tile([128, DIM], BF16, name=f"ppart{i}",
                                        tag=f"ppart{i}")
                              for i in range(4)]
            piece_rsouts = [dram.tile([32, DIM], BF16, name=f"prs{i}",
                                      tag=f"prs{i}")
                            for i in range(4)]

            # ---- persistent SBUF ----
            # one tile per DMA group, so a consumer of ktile g depends only
            # on g's DMA (a single multi-DMA tile serializes the first
            # projection behind the LAST weight DMA)
            wq_sbs = [wpool.tile([128, 4, 512], BF16, name=f"wq{q}",
                                 tag=f"wq{q}") for q in range(4)]
            wkv_sbs = [wpool.tile([128, 4, 256], BF16, name=f"wkv{q}",
                                  tag=f"wkv{q}") for q in range(4)]
            wo_sbs = [wpool.tile([128, DIM], BF16, name=f"wo{h}",
                                 tag=f"wo{h}") for h in range(4)]
            cosr_sb = wpool.tile([128, S], BF16)
            sinr_sb = wpool.tile([128, S], BF16)
            kt_sb = wpool.tile([128, S], BF16)        # K^T (kv0|kv1) full seq
            v_sb = wpool.tile([128, SEQT, 130], BF16)  # [V0|1|V1|1] per key tile
            iden_sb = wpool.tile([128, 128], BF16)     # identity
            ltneg_sb = wpool.tile([128, 128], BF16)    # MASK_NEG strictly lower

            # constants: ones columns of V_aug; identity; lower-tri mask
            nc.gpsimd.memset(v_sb[:, :, 64:65], 1.0)
            nc.gpsimd.memset(v_sb[:, :, 129:130], 1.0)
            for it in (iden_sb,):
                nc.gpsimd.memset(it[:], 1.0)
                nc.gpsimd.affine_select(
                    out=it[:], in_=it[:],
                    compare_op=mybir.AluOpType.is_equal,
                    fill=0.0, base=0,
                    pattern=[[-1, 128]], channel_multiplier=1,
                )
            nc.gpsimd.memset(ltneg_sb[:], MASK_NEG)
            nc.gpsimd.affine_select(
                out=ltneg_sb[:], in_=ltneg_sb[:],
                compare_op=mybir.AluOpType.is_ge,
                fill=0.0, base=-1,
                pattern=[[-1, 128]], channel_multiplier=1,
            )

            e_sb = wpool.tile([8, 512], BF16)          # recip expand indicator
            nc.gpsimd.memset(e_sb[:], 1.0)
            nc.gpsimd.affine_select(
                out=e_sb[:].rearrange("p (s j) -> p s j", s=8),
                in_=e_sb[:].rearrange("p (s j) -> p s j", s=8),
                compare_op=mybir.AluOpType.is_equal,
                fill=0.0, base=0,
                pattern=[[-1, 8], [0, 64]], channel_multiplier=1,
            )

            # ones row at partition 64: stationary for the tail's
            # denominator broadcast (recip lives on partition 64, no DMA)
            bc1_sb = wpool.tile([128, 64], BF16)
            nc.gpsimd.memset(bc1_sb[64:65, :], 1.0)

            # preload the exp table set so it doesn't stall the first QK
            warm = wpool.tile([128, 1], F32)
            nc.gpsimd.memset(warm[:], 0.0)
            nc.scalar.activation(warm[:], warm[:], Exp)

            with tc.tile_pool(name="xio", bufs=2) as xio, \
                 tc.tile_pool(name="asb", bufs=2) as asb, \
                 tc.tile_pool(name="bgps", bufs=2, space="PSUM") as bgps, \
                 tc.tile_pool(name="apsum", bufs=1, space="PSUM") as apsum:

                def load_weights():
                    """weights + rope tables: direct bf16 strided DMAs into
                    the persistent SBUF tiles, split over the scalar and sync
                    queues (both idle early)."""
                    nc.sync.dma_start(cosr_sb[:], cosr_d[:])
                    nc.scalar.dma_start(sinr_sb[:], sinr_d[:])
                    for q in range(4):
                        kt0 = 4 * q
                        eng = nc.scalar if q % 2 == 0 else nc.sync
                        eng.dma_start(
                            wq_sbs[q][:, :, :],
                            wq_d[kt0 * 128:(kt0 + 4) * 128, :].rearrange(
                                "(k p) n -> p k n", p=128))
                    for q in range(4):
                        kt0 = 4 * q
                        eng = nc.scalar if q % 2 == 0 else nc.sync
                        eng.dma_start(
                            wkv_sbs[q][:, :, :],
                            wkv_d[kt0 * 128:(kt0 + 4) * 128, :].rearrange(
                                "(k p) n -> p k n", p=128))
                    for h in range(4):
                        eng = nc.scalar if h % 2 == 0 else nc.sync
                        eng.dma_start(wo_sbs[h][:, :],
                                      wo_d[h * 128:(h + 1) * 128, :])

                def stage_x(c):
                    """x chunk c: 4 seq-tile bf16 loads (gpsimd); the PE
                    transposes them (emitted separately), evacs to SBUF."""
                    xfs = []
                    for tt in range(4):
                        gt = 4 * c + tt
                        xf = xio.tile([128, DIM], BF16, tag="xf", bufs=3,
                                      name="xf")
                        nc.gpsimd.dma_start(xf[:], x_d[gt * 128:(gt + 1) * 128, :])
                        xfs.append(xf)
                    xT = xio.tile([128, DT, CHUNK], BF16, tag="xT", bufs=2,
                                  name="xT")
                    return xfs, xT

                def pe_transpose(xfs, xT, tt):
                    """transpose seq-tile tt of a staged chunk into xT via the
                    tensor engine (16 [128,128] bf16 transposes, 4 evac-casts)."""
                    for g in range(4):
                        tps = bgps.tile([128, 512], BF16, tag="bg", bufs=2,
                                        name="tps")
                        for i in range(4):
                            dt = 4 * g + i
                            nc.tensor.transpose(
                                tps[:, 128 * i:128 * (i + 1)],
                                xfs[tt][:, dt * 128:(dt + 1) * 128],
                                iden_sb[:])
                        nc.vector.tensor_copy(
                            xT[:, 4 * g:4 * (g + 1), tt * 128:(tt + 1) * 128],
                            tps[:].rearrange("p (a q) -> p a q", a=4))

                def rope(c, ps, out):
                    """ps: [128, 512] f32 PSUM (per 32-quadrant: rows 0:16 = a,
                    16:32 = b); out: [128, 512] bf16 SBUF slice."""
                    cw = slice(c * CHUNK, (c + 1) * CHUNK)
                    t1 = asb.tile([128, CHUNK], BF16, tag="t1", bufs=1,
                                  name="t1")
                    psw = asb.tile([128, CHUNK], F32, tag="psw", bufs=1,
                                   name="psw")
                    t2 = asb.tile([128, CHUNK], BF16, tag="t2", bufs=1,
                                  name="t2")
                    nc.vector.tensor_mul(t1[:], ps[:], cosr_sb[:, cw])
                    nc.vector.stream_shuffle(psw[:], ps[:], SWAP_MASK)
                    nc.vector.tensor_mul(t2[:], psw[:], sinr_sb[:, cw])
                    nc.vector.tensor_add(out, t1[:], t2[:])

                def proj_qt(c, xT, qt, sp):
                    qps = bgps.tile([128, CHUNK], F32, tag="bg", bufs=2,
                                    name="qps")
                    for dt in range(DT):
                        nc.tensor.matmul(
                            qps[:],
                            wq_sbs[dt // 4][:, dt % 4,
                                            sp * 128:(sp + 1) * 128],
                            xT[:, dt, :], start=(dt == 0), stop=(dt == DT - 1))
                    rope(c, qps, qt[:, sp, :])

                def proj_kv(c, xT):
                    kps = bgps.tile([128, CHUNK], F32, tag="bg", bufs=2,
                                    name="kps")
                    for dt in range(DT):
                        nc.tensor.matmul(
                            kps[:], wkv_sbs[dt // 4][:, dt % 4, 0:128],
                            xT[:, dt, :], start=(dt == 0), stop=(dt == DT - 1))
                    rope(c, kps, kt_sb[:, c * CHUNK:(c + 1) * CHUNK])
                    # V via 512-wide movings (V^T), then PE-transpose back —
                    # avoids 64 LDW-bound 128-wide matmuls per chunk
                    vtp = bgps.tile([128, CHUNK], F32, tag="bg", bufs=2,
                                    name="vtp")
                    for dt in range(DT):
                        nc.tensor.matmul(
                            vtp[:], wkv_sbs[dt // 4][:, dt % 4, 128:256],
                            xT[:, dt, :], start=(dt == 0), stop=(dt == DT - 1))
                    vts = asb.tile([128, CHUNK], BF16, tag="vts", bufs=2,
                                   name="vts")
                    nc.vector.tensor_copy(vts[:], vtp[:])
                    vps = bgps.tile([128, CHUNK], BF16, tag="bg", bufs=2,
                                    name="vps")
                    for tt in range(4):
                        nc.tensor.transpose(
                            vps[:, tt * 128:(tt + 1) * 128],
                            vts[:, tt * 128:(tt + 1) * 128], iden_sb[:])
                    for tt in range(4):
                        gt = 4 * c + tt
                        nc.vector.tensor_copy(v_sb[:, gt, 0:64],
                                              vps[:, tt * 128:tt * 128 + 64])
                        nc.vector.tensor_copy(v_sb[:, gt, 65:129],
                                              vps[:, tt * 128 + 64:tt * 128 + 128])

                def emit_scale(pc, pstages, pdenoms, tail=False):
                    """normalize stages directly into the stacked wo
                    stationary (DVE writes partition-shifted for slot j=1)."""
                    recipf = asb.tile([8, CHUNK], F32, tag="recipf", bufs=1,
                                      name="recipf")
                    nc.vector.tensor_copy(recipf[:], pdenoms[:])
                    recip8 = asb.tile([8, CHUNK], F32, tag="recip", bufs=1,
                                      name="recip8")
                    nc.vector.reciprocal_approx_fast(recip8[:], recipf[:])
                    precipb = asb.tile([8, CHUNK], BF16, tag="recipb", bufs=1,
                                       name="recip8b")
                    nc.vector.tensor_copy(precipb[:], recip8[:])
                    sts = asb.tile([128, 4, CHUNK], BF16, tag="sts", bufs=2,
                                   name="sts")
                    for sp in range(4):
                        for j in range(2):
                            s = 2 * sp + j
                            rexp = bgps.tile([128, 512], F32, tag="bg",
                                             bufs=2, name="rexp")
                            nc.tensor.matmul(
                                rexp[0:64, :],
                                e_sb[:, 64 * s:64 * (s + 1)], precipb[:],
                                start=True, stop=True)
                            nc.vector.tensor_mul(
                                sts[64 * j:64 * (j + 1), sp, :],
                                pstages[sp][0:64, 512 * j:512 * (j + 1)],
                                rexp[0:64, :])
                    return sts

                def emit_wo(pc, sts, qs_list, dmas, tail=False):
                    pools = ([("bg", bgps), ("sps", apsum), ("aps", apsum)]
                             if tail else [("bg", bgps)])
                    gi = 0
                    for qs in qs_list:
                        for nb in range(4):
                            tag, pool = pools[gi % len(pools)]
                            gi += 1
                            wop = pool.tile(
                                [128, 512 if tag == "bg" else 1024], F32,
                                tag=tag, bufs=2 if tag != "aps" else 1,
                                name="wop")
                            for sp in range(4):
                                nc.tensor.matmul(
                                    wop[:, 0:512],
                                    sts[:, sp, qs * 128:(qs + 1) * 128],
                                    wo_sbs[sp][:, nb * 512:(nb + 1) * 512],
                                    start=(sp == 0), stop=(sp == 3))
                            ostage = asb.tile([128, 512], BF16, tag="ost",
                                              bufs=12, name="ostage")
                            nc.vector.tensor_copy(ostage[:], wop[:, 0:512])
                            peng = nc.scalar if (tail and nb % 2 == 0) else nc.sync
                            if tail:
                                tgt = piece_partials[qs][:,
                                                         nb * 512:(nb + 1) * 512]
                            else:
                                tgt = partials[pc][qs * 128:(qs + 1) * 128,
                                                   nb * 512:(nb + 1) * 512]
                            dmas.append(peng.dma_start(tgt, ostage[:]))

                def emit_cc(pc, dmas):
                    cc = nc.gpsimd.collective_compute(
                        "ReduceScatter", mybir.AluOpType.add,
                        replica_groups=GROUPS,
                        ins=[partials[pc][:, :].opt()],
                        outs=[rsouts[pc][:, :].opt()])
                    for d in dmas:
                        add_dep_helper(cc.ins, d.ins, sync=True,
                                       reason="RS waits partial DMAs")
                    cc_insts.append(cc)
                    od = nc.gpsimd.dma_start(
                        out_d[pc * 128:(pc + 1) * 128, :], rsouts[pc][:, :])
                    add_dep_helper(od.ins, cc.ins, sync=True,
                                   reason="out copy waits RS")

                def emit_cc_piece(pc, qs, dmas):
                    """tail chunk: ReduceScatter one 128-row qs block; pieces
                    post to the collective barrier early and absorb the
                    inter-core skew that a single big tail RS exposes."""
                    cc = nc.gpsimd.collective_compute(
                        "ReduceScatter", mybir.AluOpType.add,
                        replica_groups=GROUPS,
                        ins=[piece_partials[qs][:, :].opt()],
                        outs=[piece_rsouts[qs][:, :].opt()])
                    for d in dmas:
                        add_dep_helper(cc.ins, d.ins, sync=True,
                                       reason="RS piece waits partial DMAs")
                    cc_insts.append(cc)
                    od = nc.gpsimd.dma_start(
                        out_d[pc * 128 + qs * 32:pc * 128 + (qs + 1) * 32, :],
                        piece_rsouts[qs][:, :])
                    add_dep_helper(od.ins, cc.ins, sync=True,
                                   reason="out copy waits RS piece")

                # ---- fused main loop ----
                cc_insts = []
                wodmas = {}
                xbs, xT = stage_x(0)
                # dummy transposes keep the PE HAM-warm while the first x
                # chunk's DMAs land, so real work starts at full clock
                for wu in range(96):
                    wups = bgps.tile([128, 512], BF16, tag="bg", bufs=2,
                                     name="wups")
                    nc.tensor.transpose(wups[:, 0:128], iden_sb[:],
                                        iden_sb[:])
                for tt in range(4):
                    pe_transpose(xbs, xT, tt)
                load_weights()
                pending = None
                psts = {}
                nxt = None

                def attn_kts(c, qt, sp, aps, kt_lo, kt_hi):
                    # software-pipelined by one stage: AV(kt) is emitted
                    # after scores(kt+1), so the exp on the scalar engine
                    # gets a full PE slot of cover and AV never stalls
                    def emit_av(kt, pt, vs):
                        for j in range(2):
                            nc.tensor.matmul(
                                aps[0:65, 512 * j + vs:512 * j + 512],
                                v_sb[:, kt, 65 * j:65 * j + 65],
                                pt[:, 512 * j + vs:512 * j + 512],
                                start=(kt == 0), stop=(kt == 4 * c + 3))
                    prev = None
                    for kt in range(kt_lo, kt_hi):
                        vs = max(0, 128 * kt - CHUNK * c)
                        diag = kt >= 4 * c
                        spt = apsum.tile([128, 1024], F32, tag="sps",
                                         bufs=2, name="spt")
                        for j in range(2):
                            nc.tensor.matmul(
                                spt[:, 512 * j + vs:512 * j + 512],
                                kt_sb[64 * j:64 * j + 64, kt * 128:(kt + 1) * 128],
                                qt[64 * j:64 * j + 64, sp, vs:CHUNK],
                                start=True, stop=not diag)
                        if diag:
                            for j in range(2):
                                nc.tensor.matmul(
                                    spt[:, 512 * j + vs:512 * j + vs + 128],
                                    iden_sb[:], ltneg_sb[:],
                                    start=False, stop=True,
                                    skip_group_check=True)
                        pt = asb.tile([128, 1024], BF16, tag="pT", bufs=3,
                                      name="pt")
                        nc.scalar.activation(
                            pt[:].rearrange("p (h q) -> p h q", h=2)[:, :, vs:512],
                            spt[:].rearrange("p (h q) -> p h q", h=2)[:, :, vs:512],
                            Exp, scale=0.125)
                        if prev is not None:
                            emit_av(*prev)
                        prev = (kt, pt, vs)
                    emit_av(*prev)

                qt = xio.tile([128, 4, CHUNK], BF16, tag="qt", bufs=2,
                              name="qt")
                proj_qt(0, xT, qt, 0)
                proj_kv(0, xT)
                for s2 in (1, 2, 3):
                    proj_qt(0, xT, qt, s2)
                for c in range(NCH):
                    last = c == NCH - 1
                    if c + 1 < NCH:
                        nxt = stage_x(c + 1)
                    denoms8 = asb.tile([8, CHUNK], BF16, tag="denoms", bufs=1,
                                       name="denoms8")
                    stgs = []
                    if last:
                        sts3 = asb.tile([128, 4, CHUNK], BF16, tag="sts",
                                        bufs=2, name="sts3")
                    qt_next = None
                    for sp in range(4):
                        aps = apsum.tile([128, 1024], F32, tag="aps", bufs=1,
                                         name="aps")
                        attn_kts(c, qt, sp, aps, 0, 4 * c + 4)
                        stg = asb.tile([128, 1024], BF16, tag="stage", bufs=4,
                                       name="stg")
                        nc.vector.tensor_copy(stg[0:65, :], aps[0:65, :])
                        if not last:
                            for j in range(2):
                                s = 2 * sp + j
                                nc.sync.dma_start(
                                    denoms8[s:s + 1, :],
                                    stg[64:65, 512 * j:512 * (j + 1)])
                        stgs.append(stg)
                        if pending is not None:
                            ppc = pending[0]
                            if sp == 0:
                                psts[ppc] = emit_scale(*pending)
                                wodmas[ppc] = []
                                emit_wo(ppc, psts[ppc], [0, 1], wodmas[ppc])
                            elif sp == 1:
                                emit_wo(ppc, psts[ppc], [2, 3], wodmas[ppc])
                                emit_cc(ppc, wodmas[ppc])
                                pending = None
                        if last:
                            # inline per-slot-pair normalization with the
                            # reciprocal computed in place on partition 64
                            # (no cross-partition DMA on the critical tail)
                            rrec = asb.tile([128, 2 * CHUNK], F32, tag="rrec",
                                            bufs=2, name="rrec")
                            # custom-DVE recip misbehaves at base partition
                            # 64; run rows 0:65 (base 0) and use row 64 only
                            nc.vector.reciprocal_approx_fast(
                                rrec[0:65, :], aps[0:65, :])
                            rrb = asb.tile([128, 2 * CHUNK], BF16, tag="rrb",
                                           bufs=2, name="rrb")
                            nc.vector.tensor_copy(rrb[64:65, :],
                                                  rrec[64:65, :])
                            for j in range(2):
                                rexp = bgps.tile([128, 512], F32, tag="bg",
                                                 bufs=2, name="rexp")
                                nc.tensor.matmul(
                                    rexp[0:64, :],
                                    bc1_sb[64:65, :],
                                    rrb[64:65, 512 * j:512 * (j + 1)],
                                    start=True, stop=True)
                                nc.vector.tensor_mul(
                                    sts3[64 * j:64 * (j + 1), sp, :],
                                    stg[0:64, 512 * j:512 * (j + 1)],
                                    rexp[0:64, :])
                        else:
                            if sp == 0:
                                pe_transpose(nxt[0], nxt[1], 0)
                                pe_transpose(nxt[0], nxt[1], 1)
                            elif sp == 1:
                                pe_transpose(nxt[0], nxt[1], 2)
                                pe_transpose(nxt[0], nxt[1], 3)
                            elif sp == 3:
                                # pipeline the next chunk's projections into
                                # this chunk's last attention leg
                                qt_next = xio.tile([128, 4, CHUNK], BF16,
                                                   tag="qt", bufs=2, name="qt")
                                proj_qt(c + 1, nxt[1], qt_next, 0)
                                proj_kv(c + 1, nxt[1])
                                for s2 in (1, 2, 3):
                                    proj_qt(c + 1, nxt[1], qt_next, s2)
                    if last:
                        for qs in range(4):
                            piece = []
                            emit_wo(c, sts3, [qs], piece, tail=True)
                            emit_cc_piece(c, qs, piece)
                    else:
                        pending = (c, stgs, denoms8)
                        xT = nxt[1]
                        qt = qt_next

    nc.finalize()
    return nc


_NC_CACHE = None


def _get_nc():
    global _NC_CACHE
    if _NC_CACHE is None:
        _NC_CACHE = _build()
    return _NC_CACHE


def _shard_inputs(x, wq, wk, wv, wo, freqs_cos, freqs_sin):
    """Pure layout work: slice batch, pick each core's heads, permute rope
    pairs within each head, shard wo rows per core, replicate cos/sin.
    Everything is cast to bf16 host-side (the device matmuls are bf16
    anyway) to halve the input DMA bytes."""
    import ml_dtypes
    bf16 = ml_dtypes.bfloat16
    x = np.ascontiguousarray(np.asarray(x, dtype=np.float32).astype(bf16))
    wq = np.asarray(wq, dtype=np.float32).astype(bf16)
    wk = np.asarray(wk, dtype=np.float32).astype(bf16)
    wv = np.asarray(wv, dtype=np.float32).astype(bf16)
    wo = np.asarray(wo, dtype=np.float32).astype(bf16)
    cos = np.asarray(freqs_cos, dtype=np.float32)
    sin = np.asarray(freqs_sin, dtype=np.float32)

    # replicated rope tables matching the transposed Q^T/K^T row layout:
    # row r (within a 64-row slot block, w = r % 64, quadrant q2 = w // 16):
    # freq index i = (q2 // 2) * 16 + (w % 16); a-halves (q2 even) get -sin.
    cosr = np.empty((128, S), dtype=np.float32)
    sinr = np.empty((128, S), dtype=np.float32)
    for r in range(128):
        w = r % 64
        q2 = w // 16
        i = (q2 // 2) * 16 + (w % 16)
        cosr[r] = cos[:, i]
        sinr[r] = (-1.0 if q2 % 2 == 0 else 1.0) * sin[:, i]
    cosr = np.ascontiguousarray(cosr.astype(bf16))
    sinr = np.ascontiguousarray(sinr.astype(bf16))

    in_maps = []
    for core in range(N_CORES):
        b, g = core // 4, core % 4
        wq_cols = []
        wo_rows = []
        for s_ in range(8):
            h = 8 * g + SLOT_TO_LOCAL[s_]
            wq_cols.append(wq[:, 64 * h + HD_PERM])
            wo_rows.append(wo[64 * h:64 * (h + 1), :])
        wq_s = np.ascontiguousarray(np.concatenate(wq_cols, axis=1))
        wo_s = np.ascontiguousarray(np.concatenate(wo_rows, axis=0))
        wk_cols = [wk[:, 64 * (2 * g + j) + HD_PERM] for j in range(2)]
        wv_cols = wv[:, 64 * 2 * g: 64 * (2 * g + 2)]
        wkv_s = np.ascontiguousarray(
            np.concatenate(wk_cols + [wv_cols], axis=1))
        in_maps.append({
            "x": x[b], "wq": wq_s, "wkv": wkv_s, "wo": wo_s,
            "cosr": cosr, "sinr": sinr,
        })
    return in_maps


def kernel(x, wq, wk, wv, wo, freqs_cos, freqs_sin, mask=None, start_pos=0,
           **_unused):
    nc = _get_nc()
    in_maps = _shard_inputs(x, wq, wk, wv, wo, freqs_cos, freqs_sin)
    res = bass_utils.run_bass_kernel_spmd(
        nc, in_maps, core_ids=list(range(N_CORES)))
    out = np.empty((B, S, DIM), dtype=np.float32)
    for core in range(N_CORES):
        b, g = core // 4, core % 4
        co = np.asarray(res.results[core]["out"]).astype(np.float32)
        for c in range(NCH - 1):
            out[b, CHUNK * c + 128 * g: CHUNK * c + 128 * (g + 1), :] = \
                co[128 * c:128 * (c + 1), :]
        # last chunk was ReduceScattered in 4 qs pieces of 32 rows each
        c = NCH - 1
        for qs in range(4):
            r0 = CHUNK * c + 128 * qs + 32 * g
            out[b, r0:r0 + 32, :] = \
                co[128 * c + 32 * qs:128 * c + 32 * (qs + 1), :]
    return out



# revision 55
# speedup vs baseline: 1.0593x; 1.0593x over previous
"""GQA attention (B=2,S=2048,DIM=2048,H=32,KVH=8,HD=64) + RoPE, causal.

Distributed over 8 TRN2 NeuronCores: core = 4*batch + head_group.
Each core computes attention for its 8 q-heads (2 kv-heads) of one batch.
Q^T / K^T are produced directly by the projection matmuls (weights
stationary, x^T moving) so no transpose of Q/K is ever needed; RoPE is
applied in the transposed [hd, seq] layout with replicated cos/sin rows.
The causal mask is fused into the score matmul as an accumulated
(identity x lower-triangular -240) product.  The attention inner loop is
software-pipelined one stage (AV of key-tile k issues after the scores
of k+1) so the scalar-engine exp never stalls the PE.  The output
projection is computed per chunk as partial products against the core's
own 512 rows of wo, then summed + distributed with a per-chunk
ReduceScatter over per-chunk DRAM tensors (the last chunk in four
128-row pieces so the final collectives pipeline and absorb inter-core
skew).  The tail softmax denominators are inverted in place on
partition 64 and broadcast with a one-row PE matmul — no cross-partition
DMA on the critical tail.  All inputs are pre-cast to bf16 on the host
(the matmuls are bf16 anyway), weights DMA straight into per-DMA-group
SBUF tiles, and a burst of dummy transposes keeps the PE clock warm
while the first x chunk loads.
Host-side work is layout-only: weight column/row permutations, batch
split, cos/sin row replication, bf16 casts, and concatenation of
per-core outputs.
"""
import numpy as np

import concourse.bass as bass
import concourse.bacc as bacc
import concourse.tile as tile
from concourse.tile import add_dep_helper
import concourse.mybir as mybir
from concourse import bass_utils


def _ensure_axon_hooks_shim():
    """bass_utils imports antenv.axon_hooks when BASS_TRACE is set; the
    module is absent in some images. Provide a no-op shim so tracing env
    vars cannot crash the run."""
    import sys, types
    try:
        import antenv  # noqa
        if "antenv.axon_hooks" in sys.modules:
            return
        import importlib
        try:
            importlib.import_module("antenv.axon_hooks")
            return
        except ImportError:
            pass
        mod = types.ModuleType("antenv.axon_hooks")
        mod._hook = None
        mod.get_axon_ntff_profile_hook = lambda: mod._hook

        def set_axon_ntff_profile_hook(h):
            mod._hook = h
        mod.set_axon_ntff_profile_hook = set_axon_ntff_profile_hook
        sys.modules["antenv.axon_hooks"] = mod
        antenv.axon_hooks = mod
    except Exception:
        pass


_ensure_axon_hooks_shim()

F32 = mybir.dt.float32
BF16 = mybir.dt.bfloat16

B, S, DIM = 2, 2048, 2048
H, KVH, HD = 32, 8, 64
N_CORES = 8
GROUPS = [[0, 1, 2, 3], [4, 5, 6, 7]]
NCH = 4            # sequence chunks (queries) of 512
CHUNK = S // NCH   # 512
SEQT = S // 128    # 16 seq tiles
DT = DIM // 128    # 16 contraction tiles
# q-head slot order inside a core: slot s holds local q-head s//2 + 4*(s%2),
# so slot parity == local kv-head index (kv = local_head // 4).
SLOT_TO_LOCAL = [s // 2 + 4 * (s % 2) for s in range(8)]
# rope pair permutation within one head: 16-interleaved halves so the
# (a, b) cross-swap is a within-32-quadrant partition shuffle:
# [a0..a15, b0..b15, a16..a31, b16..b31] where a_i = dim 2i, b_i = dim 2i+1
HD_PERM = np.concatenate([np.arange(0, 32, 2), np.arange(1, 32, 2),
                          np.arange(32, 64, 2), np.arange(33, 64, 2)])
SWAP_MASK = list(range(16, 32)) + list(range(0, 16))
MASK_NEG = -240.0


def _build():
    nc = bacc.Bacc("TRN2", target_bir_lowering=False, debug=False,
                   num_devices=N_CORES)
    x_d = nc.dram_tensor("x", [S, DIM], BF16, kind="ExternalInput")
    wq_d = nc.dram_tensor("wq", [DIM, 512], BF16, kind="ExternalInput")
    wkv_d = nc.dram_tensor("wkv", [DIM, 256], BF16, kind="ExternalInput")
    wo_d = nc.dram_tensor("wo", [512, DIM], BF16, kind="ExternalInput")
    cosr_d = nc.dram_tensor("cosr", [128, S], BF16, kind="ExternalInput")
    sinr_d = nc.dram_tensor("sinr", [128, S], BF16, kind="ExternalInput")
    out_d = nc.dram_tensor("out", [CHUNK, DIM], BF16, kind="ExternalOutput")

    Exp = mybir.ActivationFunctionType.Exp
    Copy = mybir.ActivationFunctionType.Copy

    with tile.TileContext(nc) as tc:
        with tc.tile_pool(name="dram", bufs=1, space="DRAM") as dram, \
             tc.tile_pool(name="wpool", bufs=1) as wpool:
            # ---- DRAM scratch ----
            # one tensor per chunk (and per tail piece) so the tile dep
            # tracker never serializes chunk c+1's partial-writes behind
            # chunk c's ReduceScatter read of a shared tensor
            partials = [dram.tile([CHUNK, DIM], BF16, name=f"partial{i}",
                                  tag=f"partial{i}")
                        for i in range(NCH - 1)]
            rsouts = [dram.tile([128, DIM], BF16, name=f"rsout{i}",
                                tag=f"rsout{i}")
                      for i in range(NCH - 1)]
            piece_partials = [dram.tile([128, DIM], BF16, name=f"ppart{i}",
                                        tag=f"ppart{i}")
                              for i in range(4)]
            piece_rsouts = [dram.tile([32, DIM], BF16, name=f"prs{i}",
                                      tag=f"prs{i}")
                            for i in range(4)]

            # ---- persistent SBUF ----
            # one tile per DMA group, so a consumer of ktile g depends only
            # on g's DMA (a single multi-DMA tile serializes the first
            # projection behind the LAST weight DMA)
            wq_sbs = [wpool.tile([128, 4, 512], BF16, name=f"wq{q}",
                                 tag=f"wq{q}") for q in range(4)]
            wkv_sbs = [wpool.tile([128, 4, 256], BF16, name=f"wkv{q}",
                                  tag=f"wkv{q}") for q in range(4)]
            wo_sbs = [wpool.tile([128, DIM], BF16, name=f"wo{h}",
                                 tag=f"wo{h}") for h in range(4)]
            cosr_sb = wpool.tile([128, S], BF16)
            sinr_sb = wpool.tile([128, S], BF16)
            kt_sb = wpool.tile([128, S], BF16)        # K^T (kv0|kv1) full seq
            v_sb = wpool.tile([128, SEQT, 130], BF16)  # [V0|1|V1|1] per key tile
            iden_sb = wpool.tile([128, 128], BF16)     # identity
            ltneg_sb = wpool.tile([128, 128], BF16)    # MASK_NEG strictly lower

            # constants: ones columns of V_aug; identity; lower-tri mask
            nc.gpsimd.memset(v_sb[:, :, 64:65], 1.0)
            nc.gpsimd.memset(v_sb[:, :, 129:130], 1.0)
            for it in (iden_sb,):
                nc.gpsimd.memset(it[:], 1.0)
                nc.gpsimd.affine_select(
                    out=it[:], in_=it[:],
                    compare_op=mybir.AluOpType.is_equal,
                    fill=0.0, base=0,
                    pattern=[[-1, 128]], channel_multiplier=1,
                )
            nc.gpsimd.memset(ltneg_sb[:], MASK_NEG)
            nc.gpsimd.affine_select(
                out=ltneg_sb[:], in_=ltneg_sb[:],
                compare_op=mybir.AluOpType.is_ge,
                fill=0.0, base=-1,
                pattern=[[-1, 128]], channel_multiplier=1,
            )

            e_sb = wpool.tile([8, 512], BF16)          # recip expand indicator
            nc.gpsimd.memset(e_sb[:], 1.0)
            nc.gpsimd.affine_select(
                out=e_sb[:].rearrange("p (s j) -> p s j", s=8),
                in_=e_sb[:].rearrange("p (s j) -> p s j", s=8),
                compare_op=mybir.AluOpType.is_equal,
                fill=0.0, base=0,
                pattern=[[-1, 8], [0, 64]], channel_multiplier=1,
            )

            # ones row at partition 64: stationary for the tail's
            # denominator broadcast (recip lives on partition 64, no DMA)
            bc1_sb = wpool.tile([128, 64], BF16)
            nc.gpsimd.memset(bc1_sb[64:65, :], 1.0)

            # preload the exp table set so it doesn't stall the first QK
            warm = wpool.tile([128, 1], F32)
            nc.gpsimd.memset(warm[:], 0.0)
            nc.scalar.activation(warm[:], warm[:], Exp)

            with tc.tile_pool(name="xio", bufs=2) as xio, \
                 tc.tile_pool(name="asb", bufs=2) as asb, \
                 tc.tile_pool(name="bgps", bufs=2, space="PSUM") as bgps, \
                 tc.tile_pool(name="apsum", bufs=1, space="PSUM") as apsum:

                def load_weights():
                    """weights + rope tables: direct bf16 strided DMAs into
                    the persistent SBUF tiles, split over the scalar and sync
                    queues (both idle early)."""
                    nc.sync.dma_start(cosr_sb[:], cosr_d[:])
                    nc.scalar.dma_start(sinr_sb[:], sinr_d[:])
                    for q in range(4):
                        kt0 = 4 * q
                        eng = nc.scalar if q % 2 == 0 else nc.sync
                        eng.dma_start(
                            wq_sbs[q][:, :, :],
                            wq_d[kt0 * 128:(kt0 + 4) * 128, :].rearrange(
                                "(k p) n -> p k n", p=128))
                    for q in range(4):
                        kt0 = 4 * q
                        eng = nc.scalar if q % 2 == 0 else nc.sync
                        eng.dma_start(
                            wkv_sbs[q][:, :, :],
                            wkv_d[kt0 * 128:(kt0 + 4) * 128, :].rearrange(
                                "(k p) n -> p k n", p=128))
                    for h in range(4):
                        eng = nc.scalar if h % 2 == 0 else nc.sync
                        eng.dma_start(wo_sbs[h][:, :],
                                      wo_d[h * 128:(h + 1) * 128, :])

                def stage_x(c):
                    """x chunk c: 4 seq-tile bf16 loads (gpsimd); the PE
                    transposes them (emitted separately), evacs to SBUF."""
                    xfs = []
                    for tt in range(4):
                        gt = 4 * c + tt
                        xf = xio.tile([128, DIM], BF16, tag="xf", bufs=3,
                                      name="xf")
                        nc.gpsimd.dma_start(xf[:], x_d[gt * 128:(gt + 1) * 128, :])
                        xfs.append(xf)
                    xT = xio.tile([128, DT, CHUNK], BF16, tag="xT", bufs=2,
                                  name="xT")
                    return xfs, xT

                def pe_transpose(xfs, xT, tt):
                    """transpose seq-tile tt of a staged chunk into xT via the
                    tensor engine (16 [128,128] bf16 transposes, 4 evac-casts)."""
                    for g in range(4):
                        tps = bgps.tile([128, 512], BF16, tag="bg", bufs=2,
                                        name="tps")
                        for i in range(4):
                            dt = 4 * g + i
                            nc.tensor.transpose(
                                tps[:, 128 * i:128 * (i + 1)],
                                xfs[tt][:, dt * 128:(dt + 1) * 128],
                                iden_sb[:])
                        nc.vector.tensor_copy(
                            xT[:, 4 * g:4 * (g + 1), tt * 128:(tt + 1) * 128],
                            tps[:].rearrange("p (a q) -> p a q", a=4))

                def rope(c, ps, out):
                    """ps: [128, 512] f32 PSUM (per 32-quadrant: rows 0:16 = a,
                    16:32 = b); out: [128, 512] bf16 SBUF slice."""
                    cw = slice(c * CHUNK, (c + 1) * CHUNK)
                    t1 = asb.tile([128, CHUNK], BF16, tag="t1", bufs=1,
                                  name="t1")
                    psw = asb.tile([128, CHUNK], F32, tag="psw", bufs=1,
                                   name="psw")
                    t2 = asb.tile([128, CHUNK], BF16, tag="t2", bufs=1,
                                  name="t2")
                    nc.vector.tensor_mul(t1[:], ps[:], cosr_sb[:, cw])
                    nc.vector.stream_shuffle(psw[:], ps[:], SWAP_MASK)
                    nc.vector.tensor_mul(t2[:], psw[:], sinr_sb[:, cw])
                    nc.vector.tensor_add(out, t1[:], t2[:])

                def proj_qt(c, xT, qt, sp):
                    qps = bgps.tile([128, CHUNK], F32, tag="bg", bufs=2,
                                    name="qps")
                    for dt in range(DT):
                        nc.tensor.matmul(
                            qps[:],
                            wq_sbs[dt // 4][:, dt % 4,
                                            sp * 128:(sp + 1) * 128],
                            xT[:, dt, :], start=(dt == 0), stop=(dt == DT - 1))
                    rope(c, qps, qt[:, sp, :])

                def proj_kv(c, xT):
                    kps = bgps.tile([128, CHUNK], F32, tag="bg", bufs=2,
                                    name="kps")
                    for dt in range(DT):
                        nc.tensor.matmul(
                            kps[:], wkv_sbs[dt // 4][:, dt % 4, 0:128],
                            xT[:, dt, :], start=(dt == 0), stop=(dt == DT - 1))
                    rope(c, kps, kt_sb[:, c * CHUNK:(c + 1) * CHUNK])
                    # V via 512-wide movings (V^T), then PE-transpose back —
                    # avoids 64 LDW-bound 128-wide matmuls per chunk
                    vtp = bgps.tile([128, CHUNK], F32, tag="bg", bufs=2,
                                    name="vtp")
                    for dt in range(DT):
                        nc.tensor.matmul(
                            vtp[:], wkv_sbs[dt // 4][:, dt % 4, 128:256],
                            xT[:, dt, :], start=(dt == 0), stop=(dt == DT - 1))
                    vts = asb.tile([128, CHUNK], BF16, tag="vts", bufs=2,
                                   name="vts")
                    nc.vector.tensor_copy(vts[:], vtp[:])
                    vps = bgps.tile([128, CHUNK], BF16, tag="bg", bufs=2,
                                    name="vps")
                    for tt in range(4):
                        nc.tensor.transpose(
                            vps[:, tt * 128:(tt + 1) * 128],
                            vts[:, tt * 128:(tt + 1) * 128], iden_sb[:])
                    for tt in range(4):
                        gt = 4 * c + tt
                        nc.vector.tensor_copy(v_sb[:, gt, 0:64],
                                              vps[:, tt * 128:tt * 128 + 64])
                        nc.vector.tensor_copy(v_sb[:, gt, 65:129],
                                              vps[:, tt * 128 + 64:tt * 128 + 128])

                def emit_scale(pc, pstages, pdenoms, tail=False):
                    """normalize stages directly into the stacked wo
                    stationary (DVE writes partition-shifted for slot j=1)."""
                    recipf = asb.tile([8, CHUNK], F32, tag="recipf", bufs=1,
                                      name="recipf")
                    nc.vector.tensor_copy(recipf[:], pdenoms[:])
                    recip8 = asb.tile([8, CHUNK], F32, tag="recip", bufs=1,
                                      name="recip8")
                    nc.vector.reciprocal_approx_fast(recip8[:], recipf[:])
                    precipb = asb.tile([8, CHUNK], BF16, tag="recipb", bufs=1,
                                       name="recip8b")
                    nc.vector.tensor_copy(precipb[:], recip8[:])
                    sts = asb.tile([128, 4, CHUNK], BF16, tag="sts", bufs=2,
                                   name="sts")
                    for sp in range(4):
                        for j in range(2):
                            s = 2 * sp + j
                            rexp = bgps.tile([128, 512], F32, tag="bg",
                                             bufs=2, name="rexp")
                            nc.tensor.matmul(
                                rexp[0:64, :],
                                e_sb[:, 64 * s:64 * (s + 1)], precipb[:],
                                start=True, stop=True)
                            nc.vector.tensor_mul(
                                sts[64 * j:64 * (j + 1), sp, :],
                                pstages[sp][0:64, 512 * j:512 * (j + 1)],
                                rexp[0:64, :])
                    return sts

                def emit_wo(pc, sts, qs_list, dmas, tail=False):
                    pools = ([("bg", bgps), ("sps", apsum), ("aps", apsum)]
                             if tail else [("bg", bgps)])
                    gi = 0
                    for qs in qs_list:
                        for nb in range(4):
                            tag, pool = pools[gi % len(pools)]
                            gi += 1
                            wop = pool.tile(
                                [128, 512 if tag == "bg" else 1024], F32,
                                tag=tag, bufs=2 if tag != "aps" else 1,
                                name="wop")
                            for sp in range(4):
                                nc.tensor.matmul(
                                    wop[:, 0:512],
                                    sts[:, sp, qs * 128:(qs + 1) * 128],
                                    wo_sbs[sp][:, nb * 512:(nb + 1) * 512],
                                    start=(sp == 0), stop=(sp == 3))
                            ostage = asb.tile([128, 512], BF16, tag="ost",
                                              bufs=8, name="ostage")
                            nc.vector.tensor_copy(ostage[:], wop[:, 0:512])
                            peng = nc.scalar if (tail and nb % 2 == 0) else nc.sync
                            if tail:
                                tgt = piece_partials[qs][:,
                                                         nb * 512:(nb + 1) * 512]
                            else:
                                tgt = partials[pc][qs * 128:(qs + 1) * 128,
                                                   nb * 512:(nb + 1) * 512]
                            dmas.append(peng.dma_start(tgt, ostage[:]))

                def emit_cc(pc, dmas):
                    cc = nc.gpsimd.collective_compute(
                        "ReduceScatter", mybir.AluOpType.add,
                        replica_groups=GROUPS,
                        ins=[partials[pc][:, :].opt()],
                        outs=[rsouts[pc][:, :].opt()])
                    for d in dmas:
                        add_dep_helper(cc.ins, d.ins, sync=True,
                                       reason="RS waits partial DMAs")
                    cc_insts.append(cc)
                    od = nc.gpsimd.dma_start(
                        out_d[pc * 128:(pc + 1) * 128, :], rsouts[pc][:, :])
                    add_dep_helper(od.ins, cc.ins, sync=True,
                                   reason="out copy waits RS")

                def emit_cc_piece(pc, qs, dmas):
                    """tail chunk: ReduceScatter one 128-row qs block; pieces
                    post to the collective barrier early and absorb the
                    inter-core skew that a single big tail RS exposes."""
                    cc = nc.gpsimd.collective_compute(
                        "ReduceScatter", mybir.AluOpType.add,
                        replica_groups=GROUPS,
                        ins=[piece_partials[qs][:, :].opt()],
                        outs=[piece_rsouts[qs][:, :].opt()])
                    for d in dmas:
                        add_dep_helper(cc.ins, d.ins, sync=True,
                                       reason="RS piece waits partial DMAs")
                    cc_insts.append(cc)
                    od = nc.gpsimd.dma_start(
                        out_d[pc * 128 + qs * 32:pc * 128 + (qs + 1) * 32, :],
                        piece_rsouts[qs][:, :])
                    add_dep_helper(od.ins, cc.ins, sync=True,
                                   reason="out copy waits RS piece")

                # ---- fused main loop ----
                cc_insts = []
                wodmas = {}
                xbs, xT = stage_x(0)
                # dummy transposes keep the PE HAM-warm while the first x
                # chunk's DMAs land, so real work starts at full clock
                for wu in range(96):
                    wups = bgps.tile([128, 512], BF16, tag="bg", bufs=2,
                                     name="wups")
                    nc.tensor.transpose(wups[:, 0:128], iden_sb[:],
                                        iden_sb[:])
                for tt in range(4):
                    pe_transpose(xbs, xT, tt)
                load_weights()
                pending = None
                psts = {}
                nxt = None

                def attn_kts(c, qt, sp, aps, kt_lo, kt_hi):
                    # software-pipelined by one stage: AV(kt) is emitted
                    # after scores(kt+1), so the exp on the scalar engine
                    # gets a full PE slot of cover and AV never stalls
                    def emit_av(kt, pt, vs):
                        for j in range(2):
                            nc.tensor.matmul(
                                aps[0:65, 512 * j + vs:512 * j + 512],
                                v_sb[:, kt, 65 * j:65 * j + 65],
                                pt[:, 512 * j + vs:512 * j + 512],
                                start=(kt == 0), stop=(kt == 4 * c + 3))
                    prev = None
                    for kt in range(kt_lo, kt_hi):
                        vs = max(0, 128 * kt - CHUNK * c)
                        diag = kt >= 4 * c
                        spt = apsum.tile([128, 1024], F32, tag="sps",
                                         bufs=2, name="spt")
                        for j in range(2):
                            nc.tensor.matmul(
                                spt[:, 512 * j + vs:512 * j + 512],
                                kt_sb[64 * j:64 * j + 64, kt * 128:(kt + 1) * 128],
                                qt[64 * j:64 * j + 64, sp, vs:CHUNK],
                                start=True, stop=not diag)
                        if diag:
                            for j in range(2):
                                nc.tensor.matmul(
                                    spt[:, 512 * j + vs:512 * j + vs + 128],
                                    iden_sb[:], ltneg_sb[:],
                                    start=False, stop=True,
                                    skip_group_check=True)
                        pt = asb.tile([128, 1024], BF16, tag="pT", bufs=3,
                                      name="pt")
                        nc.scalar.activation(
                            pt[:].rearrange("p (h q) -> p h q", h=2)[:, :, vs:512],
                            spt[:].rearrange("p (h q) -> p h q", h=2)[:, :, vs:512],
                            Exp, scale=0.125)
                        if prev is not None:
                            emit_av(*prev)
                        prev = (kt, pt, vs)
                    emit_av(*prev)

                qt = xio.tile([128, 4, CHUNK], BF16, tag="qt", bufs=2,
                              name="qt")
                proj_qt(0, xT, qt, 0)
                proj_kv(0, xT)
                for s2 in (1, 2, 3):
                    proj_qt(0, xT, qt, s2)
                for c in range(NCH):
                    last = c == NCH - 1
                    if c + 1 < NCH:
                        nxt = stage_x(c + 1)
                    denoms8 = asb.tile([8, CHUNK], BF16, tag="denoms", bufs=1,
                                       name="denoms8")
                    stgs = []
                    if last:
                        sts3 = asb.tile([128, 4, CHUNK], BF16, tag="sts",
                                        bufs=2, name="sts3")
                    qt_next = None
                    for sp in range(4):
                        aps = apsum.tile([128, 1024], F32, tag="aps", bufs=1,
                                         name="aps")
                        attn_kts(c, qt, sp, aps, 0, 4 * c + 4)
                        stg = asb.tile([128, 1024], BF16, tag="stage", bufs=4,
                                       name="stg")
                        nc.vector.tensor_copy(stg[0:65, :], aps[0:65, :])
                        if not last:
                            for j in range(2):
                                s = 2 * sp + j
                                nc.sync.dma_start(
                                    denoms8[s:s + 1, :],
                                    stg[64:65, 512 * j:512 * (j + 1)])
                        stgs.append(stg)
                        if pending is not None:
                            ppc = pending[0]
                            if sp == 0:
                                psts[ppc] = emit_scale(*pending)
                                wodmas[ppc] = []
                                emit_wo(ppc, psts[ppc], [0, 1], wodmas[ppc])
                            elif sp == 1:
                                emit_wo(ppc, psts[ppc], [2, 3], wodmas[ppc])
                                emit_cc(ppc, wodmas[ppc])
                                pending = None
                        if last:
                            # inline per-slot-pair normalization with the
                            # reciprocal computed in place on partition 64
                            # (no cross-partition DMA on the critical tail)
                            rrec = asb.tile([128, 2 * CHUNK], F32, tag="rrec",
                                            bufs=2, name="rrec")
                            # custom-DVE recip misbehaves at base partition
                            # 64; run rows 0:65 (base 0) and use row 64 only
                            nc.vector.reciprocal_approx_fast(
                                rrec[0:65, :], aps[0:65, :])
                            rrb = asb.tile([128, 2 * CHUNK], BF16, tag="rrb",
                                           bufs=2, name="rrb")
                            nc.vector.tensor_copy(rrb[64:65, :],
                                                  rrec[64:65, :])
                            for j in range(2):
                                rexp = bgps.tile([128, 512], F32, tag="bg",
                                                 bufs=2, name="rexp")
                                nc.tensor.matmul(
                                    rexp[0:64, :],
                                    bc1_sb[64:65, :],
                                    rrb[64:65, 512 * j:512 * (j + 1)],
                                    start=True, stop=True)
                                nc.vector.tensor_mul(
                                    sts3[64 * j:64 * (j + 1), sp, :],
                                    stg[0:64, 512 * j:512 * (j + 1)],
                                    rexp[0:64, :])
                        else:
                            if sp == 0:
                                pe_transpose(nxt[0], nxt[1], 0)
                                pe_transpose(nxt[0], nxt[1], 1)
                            elif sp == 1:
                                pe_transpose(nxt[0], nxt[1], 2)
                                pe_transpose(nxt[0], nxt[1], 3)
                            elif sp == 3:
                                # pipeline the next chunk's projections into
                                # this chunk's last attention leg
                                qt_next = xio.tile([128, 4, CHUNK], BF16,
                                                   tag="qt", bufs=2, name="qt")
                                proj_qt(c + 1, nxt[1], qt_next, 0)
                                proj_kv(c + 1, nxt[1])
                                for s2 in (1, 2, 3):
                                    proj_qt(c + 1, nxt[1], qt_next, s2)
                    if last:
                        for qs in range(4):
                            piece = []
                            emit_wo(c, sts3, [qs], piece, tail=True)
                            emit_cc_piece(c, qs, piece)
                    else:
                        pending = (c, stgs, denoms8)
                        xT = nxt[1]
                        qt = qt_next

    nc.finalize()
    return nc


_NC_CACHE = None


def _get_nc():
    global _NC_CACHE
    if _NC_CACHE is None:
        _NC_CACHE = _build()
    return _NC_CACHE


def _shard_inputs(x, wq, wk, wv, wo, freqs_cos, freqs_sin):
    """Pure layout work: slice batch, pick each core's heads, permute rope
    pairs within each head, shard wo rows per core, replicate cos/sin.
    Everything is cast to bf16 host-side (the device matmuls are bf16
    anyway) to halve the input DMA bytes."""
    import ml_dtypes
    bf16 = ml_dtypes.bfloat16
    x = np.ascontiguousarray(np.asarray(x, dtype=np.float32).astype(bf16))
    wq = np.asarray(wq, dtype=np.float32).astype(bf16)
    wk = np.asarray(wk, dtype=np.float32).astype(bf16)
    wv = np.asarray(wv, dtype=np.float32).astype(bf16)
    wo = np.asarray(wo, dtype=np.float32).astype(bf16)
    cos = np.asarray(freqs_cos, dtype=np.float32)
    sin = np.asarray(freqs_sin, dtype=np.float32)

    # replicated rope tables matching the transposed Q^T/K^T row layout:
    # row r (within a 64-row slot block, w = r % 64, quadrant q2 = w // 16):
    # freq index i = (q2 // 2) * 16 + (w % 16); a-halves (q2 even) get -sin.
    cosr = np.empty((128, S), dtype=np.float32)
    sinr = np.empty((128, S), dtype=np.float32)
    for r in range(128):
        w = r % 64
        q2 = w // 16
        i = (q2 // 2) * 16 + (w % 16)
        cosr[r] = cos[:, i]
        sinr[r] = (-1.0 if q2 % 2 == 0 else 1.0) * sin[:, i]
    cosr = np.ascontiguousarray(cosr.astype(bf16))
    sinr = np.ascontiguousarray(sinr.astype(bf16))

    in_maps = []
    for core in range(N_CORES):
        b, g = core // 4, core % 4
        wq_cols = []
        wo_rows = []
        for s_ in range(8):
            h = 8 * g + SLOT_TO_LOCAL[s_]
            wq_cols.append(wq[:, 64 * h + HD_PERM])
            wo_rows.append(wo[64 * h:64 * (h + 1), :])
        wq_s = np.ascontiguousarray(np.concatenate(wq_cols, axis=1))
        wo_s = np.ascontiguousarray(np.concatenate(wo_rows, axis=0))
        wk_cols = [wk[:, 64 * (2 * g + j) + HD_PERM] for j in range(2)]
        wv_cols = wv[:, 64 * 2 * g: 64 * (2 * g + 2)]
        wkv_s = np.ascontiguousarray(
            np.concatenate(wk_cols + [wv_cols], axis=1))
        in_maps.append({
            "x": x[b], "wq": wq_s, "wkv": wkv_s, "wo": wo_s,
            "cosr": cosr, "sinr": sinr,
        })
    return in_maps


def kernel(x, wq, wk, wv, wo, freqs_cos, freqs_sin, mask=None, start_pos=0,
           **_unused):
    nc = _get_nc()
    in_maps = _shard_inputs(x, wq, wk, wv, wo, freqs_cos, freqs_sin)
    res = bass_utils.run_bass_kernel_spmd(
        nc, in_maps, core_ids=list(range(N_CORES)))
    out = np.empty((B, S, DIM), dtype=np.float32)
    for core in range(N_CORES):
        b, g = core // 4, core % 4
        co = np.asarray(res.results[core]["out"]).astype(np.float32)
        for c in range(NCH - 1):
            out[b, CHUNK * c + 128 * g: CHUNK * c + 128 * (g + 1), :] = \
                co[128 * c:128 * (c + 1), :]
        # last chunk was ReduceScattered in 4 qs pieces of 32 rows each
        c = NCH - 1
        for qs in range(4):
            r0 = CHUNK * c + 128 * qs + 32 * g
            out[b, r0:r0 + 32, :] = \
                co[128 * c + 32 * qs:128 * c + 32 * (qs + 1), :]
    return out



# revision 62
# speedup vs baseline: 1.1172x; 1.0546x over previous
"""GQA attention (B=2,S=2048,DIM=2048,H=32,KVH=8,HD=64) + RoPE, causal.

Distributed over 8 TRN2 NeuronCores: core = 4*batch + head_group.
Each core computes attention for its 8 q-heads (2 kv-heads) of one batch.
Q^T / K^T are produced directly by the projection matmuls (weights
stationary, x^T moving) so no transpose of Q/K is ever needed; RoPE is
applied in the transposed [hd, seq] layout with replicated cos/sin rows.
The causal mask is fused into the score matmul as an accumulated
(identity x lower-triangular -240) product.  The attention inner loop is
software-pipelined one stage (AV of key-tile k issues after the scores
of k+1) so the scalar-engine exp never stalls the PE.  The output
projection is computed per chunk as partial products against the core's
own 512 rows of wo, then summed + distributed with a per-chunk
ReduceScatter over per-chunk DRAM tensors (the last chunk in four
128-row pieces so the final collectives pipeline and absorb inter-core
skew).  The tail softmax denominators are inverted in place on
partition 64 and broadcast with a one-row PE matmul — no cross-partition
DMA on the critical tail.  All inputs are pre-cast to bf16 on the host
(the matmuls are bf16 anyway), weights DMA straight into per-DMA-group
SBUF tiles, and a burst of dummy transposes keeps the PE clock warm
while the first x chunk loads.
Host-side work is layout-only: weight column/row permutations, batch
split, cos/sin row replication, bf16 casts, and concatenation of
per-core outputs.
"""
import numpy as np

import concourse.bass as bass
import concourse.bacc as bacc
import concourse.tile as tile
from concourse.tile import add_dep_helper
import concourse.mybir as mybir
from concourse import bass_utils


def _ensure_axon_hooks_shim():
    """bass_utils imports antenv.axon_hooks when BASS_TRACE is set; the
    module is absent in some images. Provide a no-op shim so tracing env
    vars cannot crash the run."""
    import sys, types
    try:
        import antenv  # noqa
        if "antenv.axon_hooks" in sys.modules:
            return
        import importlib
        try:
            importlib.import_module("antenv.axon_hooks")
            return
        except ImportError:
            pass
        mod = types.ModuleType("antenv.axon_hooks")
        mod._hook = None
        mod.get_axon_ntff_profile_hook = lambda: mod._hook

        def set_axon_ntff_profile_hook(h):
            mod._hook = h
        mod.set_axon_ntff_profile_hook = set_axon_ntff_profile_hook
        sys.modules["antenv.axon_hooks"] = mod
        antenv.axon_hooks = mod
    except Exception:
        pass


_ensure_axon_hooks_shim()

F32 = mybir.dt.float32
BF16 = mybir.dt.bfloat16

B, S, DIM = 2, 2048, 2048
H, KVH, HD = 32, 8, 64
N_CORES = 8
GROUPS = [[0, 1, 2, 3], [4, 5, 6, 7]]
NCH = 4            # sequence chunks (queries) of 512
CHUNK = S // NCH   # 512
SEQT = S // 128    # 16 seq tiles
DT = DIM // 128    # 16 contraction tiles
# q-head slot order inside a core: slot s holds local q-head s//2 + 4*(s%2),
# so slot parity == local kv-head index (kv = local_head // 4).
SLOT_TO_LOCAL = [s // 2 + 4 * (s % 2) for s in range(8)]
# rope pair permutation within one head: 16-interleaved halves so the
# (a, b) cross-swap is a within-32-quadrant partition shuffle:
# [a0..a15, b0..b15, a16..a31, b16..b31] where a_i = dim 2i, b_i = dim 2i+1
HD_PERM = np.concatenate([np.arange(0, 32, 2), np.arange(1, 32, 2),
                          np.arange(32, 64, 2), np.arange(33, 64, 2)])
SWAP_MASK = list(range(16, 32)) + list(range(0, 16))
MASK_NEG = -240.0


def _build():
    nc = bacc.Bacc("TRN2", target_bir_lowering=False, debug=False,
                   num_devices=N_CORES)
    x_d = nc.dram_tensor("x", [S, DIM], BF16, kind="ExternalInput")
    wq_d = nc.dram_tensor("wq", [DIM, 512], BF16, kind="ExternalInput")
    wkv_d = nc.dram_tensor("wkv", [DIM, 256], BF16, kind="ExternalInput")
    wo_d = nc.dram_tensor("wo", [512, DIM], BF16, kind="ExternalInput")
    cosr_d = nc.dram_tensor("cosr", [128, S], BF16, kind="ExternalInput")
    sinr_d = nc.dram_tensor("sinr", [128, S], BF16, kind="ExternalInput")
    out_d = nc.dram_tensor("out", [CHUNK, DIM], BF16, kind="ExternalOutput")

    Exp = mybir.ActivationFunctionType.Exp
    Copy = mybir.ActivationFunctionType.Copy

    with tile.TileContext(nc) as tc:
        with tc.tile_pool(name="dram", bufs=1, space="DRAM") as dram, \
             tc.tile_pool(name="wpool", bufs=1) as wpool:
            # ---- DRAM scratch ----
            # one tensor per chunk (and per tail piece) so the tile dep
            # tracker never serializes chunk c+1's partial-writes behind
            # chunk c's ReduceScatter read of a shared tensor
            partials = [dram.tile([CHUNK, DIM], BF16, name=f"partial{i}",
                                  tag=f"partial{i}")
                        for i in range(NCH - 1)]
            rsouts = [dram.tile([128, DIM], BF16, name=f"rsout{i}",
                                tag=f"rsout{i}")
                      for i in range(NCH - 1)]
            piece_partials = [dram.tile([256, DIM], BF16, name=f"ppart{i}",
                                        tag=f"ppart{i}")
                              for i in range(2)]
            piece_rsouts = [dram.tile([64, DIM], BF16, name=f"prs{i}",
                                      tag=f"prs{i}")
                            for i in range(2)]

            # ---- persistent SBUF ----
            # one tile per DMA group, so a consumer of ktile g depends only
            # on g's DMA (a single multi-DMA tile serializes the first
            # projection behind the LAST weight DMA)
            wq_sbs = [wpool.tile([128, 4, 512], BF16, name=f"wq{q}",
                                 tag=f"wq{q}") for q in range(4)]
            wkv_sbs = [wpool.tile([128, 4, 256], BF16, name=f"wkv{q}",
                                  tag=f"wkv{q}") for q in range(4)]
            wo_sbs = [wpool.tile([128, DIM], BF16, name=f"wo{h}",
                                 tag=f"wo{h}") for h in range(4)]
            cosr_sb = wpool.tile([128, S], BF16)
            sinr_sb = wpool.tile([128, S], BF16)
            kt_sb = wpool.tile([128, S], BF16)        # K^T (kv0|kv1) full seq
            v_sb = wpool.tile([128, SEQT, 130], BF16)  # [V0|1|V1|1] per key tile
            iden_sb = wpool.tile([128, 128], BF16)     # identity
            ltneg_sb = wpool.tile([128, 128], BF16)    # MASK_NEG strictly lower

            # constants: ones columns of V_aug; identity; lower-tri mask
            nc.gpsimd.memset(v_sb[:, :, 64:65], 1.0)
            nc.gpsimd.memset(v_sb[:, :, 129:130], 1.0)
            for it in (iden_sb,):
                nc.gpsimd.memset(it[:], 1.0)
                nc.gpsimd.affine_select(
                    out=it[:], in_=it[:],
                    compare_op=mybir.AluOpType.is_equal,
                    fill=0.0, base=0,
                    pattern=[[-1, 128]], channel_multiplier=1,
                )
            nc.gpsimd.memset(ltneg_sb[:], MASK_NEG)
            nc.gpsimd.affine_select(
                out=ltneg_sb[:], in_=ltneg_sb[:],
                compare_op=mybir.AluOpType.is_ge,
                fill=0.0, base=-1,
                pattern=[[-1, 128]], channel_multiplier=1,
            )

            e_sb = wpool.tile([8, 512], BF16)          # recip expand indicator
            nc.gpsimd.memset(e_sb[:], 1.0)
            nc.gpsimd.affine_select(
                out=e_sb[:].rearrange("p (s j) -> p s j", s=8),
                in_=e_sb[:].rearrange("p (s j) -> p s j", s=8),
                compare_op=mybir.AluOpType.is_equal,
                fill=0.0, base=0,
                pattern=[[-1, 8], [0, 64]], channel_multiplier=1,
            )

            # ones row at partition 64: stationary for the tail's
            # denominator broadcast (recip lives on partition 64, no DMA)
            bc1_sb = wpool.tile([128, 64], BF16)
            nc.gpsimd.memset(bc1_sb[64:65, :], 1.0)

            # preload the exp table set so it doesn't stall the first QK
            warm = wpool.tile([128, 1], F32)
            nc.gpsimd.memset(warm[:], 0.0)
            nc.scalar.activation(warm[:], warm[:], Exp)

            with tc.tile_pool(name="xio", bufs=2) as xio, \
                 tc.tile_pool(name="asb", bufs=2) as asb, \
                 tc.tile_pool(name="bgps", bufs=2, space="PSUM") as bgps, \
                 tc.tile_pool(name="apsum", bufs=1, space="PSUM") as apsum:

                def load_weights():
                    """weights + rope tables: direct bf16 strided DMAs into
                    the persistent SBUF tiles, split over the scalar and sync
                    queues (both idle early)."""
                    nc.sync.dma_start(cosr_sb[:], cosr_d[:])
                    nc.scalar.dma_start(sinr_sb[:], sinr_d[:])
                    for q in range(4):
                        kt0 = 4 * q
                        eng = nc.scalar if q % 2 == 0 else nc.sync
                        eng.dma_start(
                            wq_sbs[q][:, :, :],
                            wq_d[kt0 * 128:(kt0 + 4) * 128, :].rearrange(
                                "(k p) n -> p k n", p=128))
                    for q in range(4):
                        kt0 = 4 * q
                        eng = nc.scalar if q % 2 == 0 else nc.sync
                        eng.dma_start(
                            wkv_sbs[q][:, :, :],
                            wkv_d[kt0 * 128:(kt0 + 4) * 128, :].rearrange(
                                "(k p) n -> p k n", p=128))
                    for h in range(4):
                        eng = nc.scalar if h % 2 == 0 else nc.sync
                        eng.dma_start(wo_sbs[h][:, :],
                                      wo_d[h * 128:(h + 1) * 128, :])

                def stage_x(c):
                    """x chunk c: 4 seq-tile bf16 loads; the PE transposes
                    them (emitted separately), evacs to SBUF. Chunk 0 loads
                    ride the low-latency HWDGE queues ahead of the weights;
                    later chunks use gpsimd to keep sync/scalar free."""
                    xfs = []
                    for tt in range(4):
                        gt = 4 * c + tt
                        xf = xio.tile([128, DIM], BF16, tag="xf", bufs=3,
                                      name="xf")
                        if c == 0:
                            eng = nc.sync if tt % 2 == 0 else nc.scalar
                        else:
                            eng = nc.gpsimd
                        eng.dma_start(xf[:], x_d[gt * 128:(gt + 1) * 128, :])
                        xfs.append(xf)
                    xT = xio.tile([128, DT, CHUNK], BF16, tag="xT", bufs=2,
                                  name="xT")
                    return xfs, xT

                def pe_transpose(xfs, xT, tt):
                    """transpose seq-tile tt of a staged chunk into xT via the
                    tensor engine (16 [128,128] bf16 transposes, 4 evac-casts)."""
                    for g in range(4):
                        tps = bgps.tile([128, 512], BF16, tag="bg", bufs=2,
                                        name="tps")
                        for i in range(4):
                            dt = 4 * g + i
                            nc.tensor.transpose(
                                tps[:, 128 * i:128 * (i + 1)],
                                xfs[tt][:, dt * 128:(dt + 1) * 128],
                                iden_sb[:])
                        nc.vector.tensor_copy(
                            xT[:, 4 * g:4 * (g + 1), tt * 128:(tt + 1) * 128],
                            tps[:].rearrange("p (a q) -> p a q", a=4))

                def rope(c, ps, out):
                    """ps: [128, 512] f32 PSUM (per 32-quadrant: rows 0:16 = a,
                    16:32 = b); out: [128, 512] bf16 SBUF slice."""
                    cw = slice(c * CHUNK, (c + 1) * CHUNK)
                    t1 = asb.tile([128, CHUNK], BF16, tag="t1", bufs=1,
                                  name="t1")
                    psw = asb.tile([128, CHUNK], F32, tag="psw", bufs=1,
                                   name="psw")
                    t2 = asb.tile([128, CHUNK], BF16, tag="t2", bufs=1,
                                  name="t2")
                    nc.vector.tensor_mul(t1[:], ps[:], cosr_sb[:, cw])
                    nc.vector.stream_shuffle(psw[:], ps[:], SWAP_MASK)
                    nc.vector.tensor_mul(t2[:], psw[:], sinr_sb[:, cw])
                    nc.vector.tensor_add(out, t1[:], t2[:])

                def proj_qt(c, xT, qt, sp):
                    qps = bgps.tile([128, CHUNK], F32, tag="bg", bufs=2,
                                    name="qps")
                    for dt in range(DT):
                        nc.tensor.matmul(
                            qps[:],
                            wq_sbs[dt // 4][:, dt % 4,
                                            sp * 128:(sp + 1) * 128],
                            xT[:, dt, :], start=(dt == 0), stop=(dt == DT - 1))
                    rope(c, qps, qt[:, sp, :])

                def proj_kv(c, xT):
                    kps = bgps.tile([128, CHUNK], F32, tag="bg", bufs=2,
                                    name="kps")
                    for dt in range(DT):
                        nc.tensor.matmul(
                            kps[:], wkv_sbs[dt // 4][:, dt % 4, 0:128],
                            xT[:, dt, :], start=(dt == 0), stop=(dt == DT - 1))
                    rope(c, kps, kt_sb[:, c * CHUNK:(c + 1) * CHUNK])
                    # V via 512-wide movings (V^T), then PE-transpose back —
                    # avoids 64 LDW-bound 128-wide matmuls per chunk
                    vtp = bgps.tile([128, CHUNK], F32, tag="bg", bufs=2,
                                    name="vtp")
                    for dt in range(DT):
                        nc.tensor.matmul(
                            vtp[:], wkv_sbs[dt // 4][:, dt % 4, 128:256],
                            xT[:, dt, :], start=(dt == 0), stop=(dt == DT - 1))
                    vts = asb.tile([128, CHUNK], BF16, tag="vts", bufs=2,
                                   name="vts")
                    nc.vector.tensor_copy(vts[:], vtp[:])
                    vps = bgps.tile([128, CHUNK], BF16, tag="bg", bufs=2,
                                    name="vps")
                    for tt in range(4):
                        nc.tensor.transpose(
                            vps[:, tt * 128:(tt + 1) * 128],
                            vts[:, tt * 128:(tt + 1) * 128], iden_sb[:])
                    for tt in range(4):
                        gt = 4 * c + tt
                        nc.vector.tensor_copy(v_sb[:, gt, 0:64],
                                              vps[:, tt * 128:tt * 128 + 64])
                        nc.vector.tensor_copy(v_sb[:, gt, 65:129],
                                              vps[:, tt * 128 + 64:tt * 128 + 128])

                def emit_scale(pc, pstages, pdenoms, tail=False):
                    """normalize stages directly into the stacked wo
                    stationary (DVE writes partition-shifted for slot j=1)."""
                    recipf = asb.tile([8, CHUNK], F32, tag="recipf", bufs=1,
                                      name="recipf")
                    nc.vector.tensor_copy(recipf[:], pdenoms[:])
                    recip8 = asb.tile([8, CHUNK], F32, tag="recip", bufs=1,
                                      name="recip8")
                    nc.vector.reciprocal_approx_fast(recip8[:], recipf[:])
                    precipb = asb.tile([8, CHUNK], BF16, tag="recipb", bufs=1,
                                       name="recip8b")
                    nc.vector.tensor_copy(precipb[:], recip8[:])
                    sts = asb.tile([128, 4, CHUNK], BF16, tag="sts", bufs=2,
                                   name="sts")
                    for sp in range(4):
                        for j in range(2):
                            s = 2 * sp + j
                            rexp = bgps.tile([128, 512], F32, tag="bg",
                                             bufs=2, name="rexp")
                            nc.tensor.matmul(
                                rexp[0:64, :],
                                e_sb[:, 64 * s:64 * (s + 1)], precipb[:],
                                start=True, stop=True)
                            nc.vector.tensor_mul(
                                sts[64 * j:64 * (j + 1), sp, :],
                                pstages[sp][0:64, 512 * j:512 * (j + 1)],
                                rexp[0:64, :])
                    return sts

                def emit_wo(pc, sts, qs_list, dmas, tail=False):
                    pools = ([("bg", bgps), ("sps", apsum), ("aps", apsum)]
                             if tail else [("bg", bgps)])
                    gi = 0
                    for qs in qs_list:
                        for nb in range(4):
                            tag, pool = pools[gi % len(pools)]
                            gi += 1
                            wop = pool.tile(
                                [128, 512 if tag == "bg" else 1024], F32,
                                tag=tag, bufs=2 if tag != "aps" else 1,
                                name="wop")
                            for sp in range(4):
                                nc.tensor.matmul(
                                    wop[:, 0:512],
                                    sts[:, sp, qs * 128:(qs + 1) * 128],
                                    wo_sbs[sp][:, nb * 512:(nb + 1) * 512],
                                    start=(sp == 0), stop=(sp == 3))
                            ostage = asb.tile([128, 512], BF16, tag="ost",
                                              bufs=8, name="ostage")
                            nc.vector.tensor_copy(ostage[:], wop[:, 0:512])
                            peng = nc.scalar if (tail and nb % 2 == 0) else nc.sync
                            if tail:
                                tgt = piece_partials[qs // 2][
                                    (qs % 2) * 128:(qs % 2 + 1) * 128,
                                    nb * 512:(nb + 1) * 512]
                            else:
                                tgt = partials[pc][qs * 128:(qs + 1) * 128,
                                                   nb * 512:(nb + 1) * 512]
                            dmas.append(peng.dma_start(tgt, ostage[:]))

                def emit_cc(pc, dmas):
                    cc = nc.gpsimd.collective_compute(
                        "ReduceScatter", mybir.AluOpType.add,
                        replica_groups=GROUPS,
                        ins=[partials[pc][:, :].opt()],
                        outs=[rsouts[pc][:, :].opt()])
                    for d in dmas:
                        add_dep_helper(cc.ins, d.ins, sync=True,
                                       reason="RS waits partial DMAs")
                    cc_insts.append(cc)
                    od = nc.gpsimd.dma_start(
                        out_d[pc * 128:(pc + 1) * 128, :], rsouts[pc][:, :])
                    add_dep_helper(od.ins, cc.ins, sync=True,
                                   reason="out copy waits RS")

                def emit_cc_piece(pc, h, dmas):
                    """tail chunk: ReduceScatter one 256-row half; two pieces
                    post to the collective barrier early (absorbing inter-core
                    skew) while paying the per-collective handshake only
                    twice."""
                    cc = nc.gpsimd.collective_compute(
                        "ReduceScatter", mybir.AluOpType.add,
                        replica_groups=GROUPS,
                        ins=[piece_partials[h][:, :].opt()],
                        outs=[piece_rsouts[h][:, :].opt()])
                    for d in dmas:
                        add_dep_helper(cc.ins, d.ins, sync=True,
                                       reason="RS piece waits partial DMAs")
                    cc_insts.append(cc)
                    od = nc.gpsimd.dma_start(
                        out_d[pc * 128 + h * 64:pc * 128 + (h + 1) * 64, :],
                        piece_rsouts[h][:, :])
                    add_dep_helper(od.ins, cc.ins, sync=True,
                                   reason="out copy waits RS piece")

                # ---- fused main loop ----
                cc_insts = []
                wodmas = {}
                xbs, xT = stage_x(0)
                # dummy transposes keep the PE HAM-warm while the first x
                # chunk's DMAs land, so real work starts at full clock
                for wu in range(64):
                    wups = bgps.tile([128, 512], BF16, tag="bg", bufs=2,
                                     name="wups")
                    nc.tensor.transpose(wups[:, 0:128], iden_sb[:],
                                        iden_sb[:])
                for tt in range(4):
                    pe_transpose(xbs, xT, tt)
                load_weights()
                pending = None
                psts = {}
                nxt = None

                def attn_kts(c, qt, sp, aps, kt_lo, kt_hi):
                    # software-pipelined by one stage: AV(kt) is emitted
                    # after scores(kt+1), so the exp on the scalar engine
                    # gets a full PE slot of cover and AV never stalls
                    def emit_av(kt, pt, vs):
                        for j in range(2):
                            nc.tensor.matmul(
                                aps[0:65, 512 * j + vs:512 * j + 512],
                                v_sb[:, kt, 65 * j:65 * j + 65],
                                pt[:, 512 * j + vs:512 * j + 512],
                                start=(kt == 0), stop=(kt == 4 * c + 3))
                    prev = None
                    for kt in range(kt_lo, kt_hi):
                        vs = max(0, 128 * kt - CHUNK * c)
                        diag = kt >= 4 * c
                        spt = apsum.tile([128, 1024], F32, tag="sps",
                                         bufs=2, name="spt")
                        for j in range(2):
                            nc.tensor.matmul(
                                spt[:, 512 * j + vs:512 * j + 512],
                                kt_sb[64 * j:64 * j + 64, kt * 128:(kt + 1) * 128],
                                qt[64 * j:64 * j + 64, sp, vs:CHUNK],
                                start=True, stop=not diag)
                        if diag:
                            for j in range(2):
                                nc.tensor.matmul(
                                    spt[:, 512 * j + vs:512 * j + vs + 128],
                                    iden_sb[:], ltneg_sb[:],
                                    start=False, stop=True,
                                    skip_group_check=True)
                        pt = asb.tile([128, 1024], BF16, tag="pT", bufs=3,
                                      name="pt")
                        nc.scalar.activation(
                            pt[:].rearrange("p (h q) -> p h q", h=2)[:, :, vs:512],
                            spt[:].rearrange("p (h q) -> p h q", h=2)[:, :, vs:512],
                            Exp, scale=0.125)
                        if prev is not None:
                            emit_av(*prev)
                        prev = (kt, pt, vs)
                    emit_av(*prev)

                qt = xio.tile([128, 4, CHUNK], BF16, tag="qt", bufs=2,
                              name="qt")
                proj_qt(0, xT, qt, 0)
                proj_kv(0, xT)
                for s2 in (1, 2, 3):
                    proj_qt(0, xT, qt, s2)
                for c in range(NCH):
                    last = c == NCH - 1
                    if c + 1 < NCH:
                        nxt = stage_x(c + 1)
                    denoms8 = asb.tile([8, CHUNK], BF16, tag="denoms", bufs=1,
                                       name="denoms8")
                    stgs = []
                    if last:
                        sts3 = asb.tile([128, 4, CHUNK], BF16, tag="sts",
                                        bufs=2, name="sts3")
                    qt_next = None
                    for sp in range(4):
                        aps = apsum.tile([128, 1024], F32, tag="aps", bufs=1,
                                         name="aps")
                        attn_kts(c, qt, sp, aps, 0, 4 * c + 4)
                        stg = asb.tile([128, 1024], BF16, tag="stage", bufs=4,
                                       name="stg")
                        nc.vector.tensor_copy(stg[0:65, :], aps[0:65, :])
                        if not last:
                            for j in range(2):
                                s = 2 * sp + j
                                nc.sync.dma_start(
                                    denoms8[s:s + 1, :],
                                    stg[64:65, 512 * j:512 * (j + 1)])
                        stgs.append(stg)
                        if pending is not None:
                            ppc = pending[0]
                            if sp == 0:
                                psts[ppc] = emit_scale(*pending)
                                wodmas[ppc] = []
                                emit_wo(ppc, psts[ppc], [0, 1], wodmas[ppc])
                            elif sp == 1:
                                emit_wo(ppc, psts[ppc], [2, 3], wodmas[ppc])
                                emit_cc(ppc, wodmas[ppc])
                                pending = None
                        if last:
                            # inline per-slot-pair normalization with the
                            # reciprocal computed in place on partition 64
                            # (no cross-partition DMA on the critical tail)
                            rrec = asb.tile([128, 2 * CHUNK], F32, tag="rrec",
                                            bufs=2, name="rrec")
                            # custom-DVE recip misbehaves at base partition
                            # 64; run rows 0:65 (base 0) and use row 64 only
                            nc.vector.reciprocal_approx_fast(
                                rrec[0:65, :], aps[0:65, :])
                            rrb = asb.tile([128, 2 * CHUNK], BF16, tag="rrb",
                                           bufs=2, name="rrb")
                            nc.vector.tensor_copy(rrb[64:65, :],
                                                  rrec[64:65, :])
                            for j in range(2):
                                rexp = bgps.tile([128, 512], F32, tag="bg",
                                                 bufs=2, name="rexp")
                                nc.tensor.matmul(
                                    rexp[0:64, :],
                                    bc1_sb[64:65, :],
                                    rrb[64:65, 512 * j:512 * (j + 1)],
                                    start=True, stop=True)
                                nc.vector.tensor_mul(
                                    sts3[64 * j:64 * (j + 1), sp, :],
                                    stg[0:64, 512 * j:512 * (j + 1)],
                                    rexp[0:64, :])
                        else:
                            if sp == 0:
                                pe_transpose(nxt[0], nxt[1], 0)
                                pe_transpose(nxt[0], nxt[1], 1)
                            elif sp == 1:
                                pe_transpose(nxt[0], nxt[1], 2)
                                pe_transpose(nxt[0], nxt[1], 3)
                            elif sp == 3:
                                # pipeline the next chunk's projections into
                                # this chunk's last attention leg
                                qt_next = xio.tile([128, 4, CHUNK], BF16,
                                                   tag="qt", bufs=2, name="qt")
                                proj_qt(c + 1, nxt[1], qt_next, 0)
                                proj_kv(c + 1, nxt[1])
                                for s2 in (1, 2, 3):
                                    proj_qt(c + 1, nxt[1], qt_next, s2)
                    if last:
                        for h in range(2):
                            piece = []
                            emit_wo(c, sts3, [2 * h], piece, tail=True)
                            emit_wo(c, sts3, [2 * h + 1], piece, tail=True)
                            emit_cc_piece(c, h, piece)
                    else:
                        pending = (c, stgs, denoms8)
                        xT = nxt[1]
                        qt = qt_next

    nc.finalize()
    return nc


_NC_CACHE = None


def _get_nc():
    global _NC_CACHE
    if _NC_CACHE is None:
        _NC_CACHE = _build()
    return _NC_CACHE


def _shard_inputs(x, wq, wk, wv, wo, freqs_cos, freqs_sin):
    """Pure layout work: slice batch, pick each core's heads, permute rope
    pairs within each head, shard wo rows per core, replicate cos/sin.
    Everything is cast to bf16 host-side (the device matmuls are bf16
    anyway) to halve the input DMA bytes."""
    import ml_dtypes
    bf16 = ml_dtypes.bfloat16
    x = np.ascontiguousarray(np.asarray(x, dtype=np.float32).astype(bf16))
    wq = np.asarray(wq, dtype=np.float32).astype(bf16)
    wk = np.asarray(wk, dtype=np.float32).astype(bf16)
    wv = np.asarray(wv, dtype=np.float32).astype(bf16)
    wo = np.asarray(wo, dtype=np.float32).astype(bf16)
    cos = np.asarray(freqs_cos, dtype=np.float32)
    sin = np.asarray(freqs_sin, dtype=np.float32)

    # replicated rope tables matching the transposed Q^T/K^T row layout:
    # row r (within a 64-row slot block, w = r % 64, quadrant q2 = w // 16):
    # freq index i = (q2 // 2) * 16 + (w % 16); a-halves (q2 even) get -sin.
    cosr = np.empty((128, S), dtype=np.float32)
    sinr = np.empty((128, S), dtype=np.float32)
    for r in range(128):
        w = r % 64
        q2 = w // 16
        i = (q2 // 2) * 16 + (w % 16)
        cosr[r] = cos[:, i]
        sinr[r] = (-1.0 if q2 % 2 == 0 else 1.0) * sin[:, i]
    cosr = np.ascontiguousarray(cosr.astype(bf16))
    sinr = np.ascontiguousarray(sinr.astype(bf16))

    in_maps = []
    for core in range(N_CORES):
        b, g = core // 4, core % 4
        wq_cols = []
        wo_rows = []
        for s_ in range(8):
            h = 8 * g + SLOT_TO_LOCAL[s_]
            wq_cols.append(wq[:, 64 * h + HD_PERM])
            wo_rows.append(wo[64 * h:64 * (h + 1), :])
        wq_s = np.ascontiguousarray(np.concatenate(wq_cols, axis=1))
        wo_s = np.ascontiguousarray(np.concatenate(wo_rows, axis=0))
        wk_cols = [wk[:, 64 * (2 * g + j) + HD_PERM] for j in range(2)]
        wv_cols = wv[:, 64 * 2 * g: 64 * (2 * g + 2)]
        wkv_s = np.ascontiguousarray(
            np.concatenate(wk_cols + [wv_cols], axis=1))
        in_maps.append({
            "x": x[b], "wq": wq_s, "wkv": wkv_s, "wo": wo_s,
            "cosr": cosr, "sinr": sinr,
        })
    return in_maps


def kernel(x, wq, wk, wv, wo, freqs_cos, freqs_sin, mask=None, start_pos=0,
           **_unused):
    nc = _get_nc()
    in_maps = _shard_inputs(x, wq, wk, wv, wo, freqs_cos, freqs_sin)
    res = bass_utils.run_bass_kernel_spmd(
        nc, in_maps, core_ids=list(range(N_CORES)))
    out = np.empty((B, S, DIM), dtype=np.float32)
    for core in range(N_CORES):
        b, g = core // 4, core % 4
        co = np.asarray(res.results[core]["out"]).astype(np.float32)
        for c in range(NCH - 1):
            out[b, CHUNK * c + 128 * g: CHUNK * c + 128 * (g + 1), :] = \
                co[128 * c:128 * (c + 1), :]
        # last chunk was ReduceScattered in 2 halves of 64 rows each
        c = NCH - 1
        for h in range(2):
            r0 = CHUNK * c + 256 * h + 64 * g
            out[b, r0:r0 + 64, :] = \
                co[128 * c + 64 * h:128 * c + 64 * (h + 1), :]
    return out

